# revision 10
# baseline (speedup 1.0000x reference)
"""Trainium2 Bass kernel for nn_AMLNeuralNetwork3D (dense_mlp).

Strategy v2: layer 1 is ROW-parallel (each core contracts over its own
1024 input features, producing a [8192, batch] partial), so the PE can
start real matmuls ~15us into the kernel with ZERO preceding
communication -- the launch barrier and the first gather no longer gate
compute.  Partials are summed with a bf16 ReduceScatter (feature halves),
relu+bias applied on the scattered slices, then the halves are
AllGather'd (exactly the baseline's gath0h layout).  Layers 2 and 3 stay
column-parallel: L2 consumes the gathered halves (W2 rows permuted into
half-major order), AllGather, L3 consumes the full gather and writes the
per-core output slice.

Activations stay feature-major [features, batch] on chip; weights are
pre-arranged on host so every DMA is wide/contiguous.  Compute in bf16
(fp32 PSUM accumulation).
"""

import sys

if "/opt/trn_rl_repo" not in sys.path:
    sys.path.insert(0, "/opt/trn_rl_repo")

import numpy as np
import ml_dtypes

N_CORES = 8
G = 8192          # genes / features
B = 1024          # batch
L = 4             # levels
GS = G // N_CORES # per-core feature slice (1024)
NB = 512          # batch chunk (one PSUM bank at fp32)
NCHUNK = B // NB  # 2
GT = GS // 128    # gene tiles per core slice (8)
KT = G // 128     # contraction tiles (64)

BF16 = ml_dtypes.bfloat16

_compiled = {}

# half-major feature order: [core0 f0:512, core1 f1024:1536, ...] then the
# second halves.  This is both the ReduceScatter row order of the L1
# partials and the AllGather'd h1 row order consumed by L2.
_PERM_HALVES = np.concatenate(
    [np.arange(r * 1024, r * 1024 + 512) for r in range(8)]
    + [np.arange(r * 1024 + 512, (r + 1) * 1024) for r in range(8)]
)


def _build_graph():
    from concourse import bacc, tile
    import concourse.mybir as mybir

    fp32 = mybir.dt.float32
    bf16 = mybir.dt.bfloat16
    Relu = mybir.ActivationFunctionType.Relu
    Copy = mybir.ActivationFunctionType.Copy
    mult = mybir.AluOpType.mult
    add = mybir.AluOpType.add
    bypass = mybir.AluOpType.bypass

    nc = bacc.Bacc(None, target_bir_lowering=False, num_devices=N_CORES)

    # ---- parameters (per-core shards; same graph on all cores) ----
    x_p = nc.declare_dram_parameter("x", [L, GS, B], bf16, isOutput=False)
    # per-feature scalars: cols 0..3 = W_local, 4 = b_local, 5..7 = b1..b3
    scal_p = nc.declare_dram_parameter("scal", [GS, 8], fp32, isOutput=False)
    # L1 row-parallel weights: [own 1024 in-feats, 8192 out-feats in
    # half-major (_PERM_HALVES) order]
    w1rp_p = nc.declare_dram_parameter("w1rp", [GS, G], bf16, isOutput=False)
    # L2: [8192 in-feats in half-major order, own 1024 out-feats]
    w2t_p = nc.declare_dram_parameter("w2t", [G, GS], bf16, isOutput=False)
    # L3: [8192 in-feats plain order, own 1024 out-feats]
    w3t_p = nc.declare_dram_parameter("w3t", [G, GS], bf16, isOutput=False)
    out_p = nc.declare_dram_parameter("out", [GS, B], fp32, isOutput=True)

    rg = [list(range(N_CORES))]

    with tile.TileContext(nc) as tc:
        with (
            tc.tile_pool(name="dram", bufs=1, space="DRAM") as dram,
            tc.tile_pool(name="scal", bufs=GT) as spool,
            tc.tile_pool(name="xin", bufs=16) as xpool,
            tc.tile_pool(name="loc", bufs=10) as lpool,
            tc.tile_pool(name="h0", bufs=16) as h0pool,
            tc.tile_pool(name="hin", bufs=28) as hpool,
            tc.tile_pool(name="wblk", bufs=24) as wpool,
            tc.tile_pool(name="evic", bufs=12) as epool,
            tc.tile_pool(name="hout", bufs=8) as opool,
            tc.tile_pool(name="psum", bufs=8, space="PSUM") as ppool,
        ):
            # L1 partial sums, per chunk, split in half-major halves
            part = [
                [
                    dram.tile([G // 2, NB], bf16, name=f"part_{j}_{a}",
                              tag=f"part_{j}_{a}")
                    for a in range(2)
                ]
                for j in range(NCHUNK)
            ]
            # ReduceScatter outputs: own [512, NB] summed partial slices
            rsout = [
                [
                    dram.tile([GS // 2, NB], bf16, name=f"rso_{j}_{a}",
                              tag=f"rso_{j}_{a}")
                    for a in range(2)
                ]
                for j in range(NCHUNK)
            ]
            # relu'd h1 own slices and their gathers (baseline gath0h layout)
            slc1h = [
                [
                    dram.tile([GS // 2, NB], bf16, name=f"slc1h_{j}_{a}",
                              tag=f"slc1h_{j}_{a}")
                    for a in range(2)
                ]
                for j in range(NCHUNK)
            ]
            gath1h = [
                [
                    dram.tile([G // 2, NB], bf16, name=f"g1h_{j}_{a}",
                              tag=f"g1h_{j}_{a}", addr_space="Shared")
                    for a in range(2)
                ]
                for j in range(NCHUNK)
            ]
            # L2 -> L3 transition
            slc2 = [
                dram.tile([GS, NB], bf16, name=f"slc2_{j}", tag=f"slc2_{j}")
                for j in range(NCHUNK)
            ]
            gath2 = [
                dram.tile([G, NB], bf16, name=f"g2_{j}", tag=f"g2_{j}",
                          addr_space="Shared")
                for j in range(NCHUNK)
            ]

            # per-feature scalar tiles, persistent
            sc = []
            for gt in range(GT):
                s = spool.tile([128, 8], fp32, name=f"sc{gt}", tag="sc")
                nc.sync.dma_start(s[:], scal_p[gt * 128 : (gt + 1) * 128, :])
                sc.append(s)

            h0 = [[None] * GT for _ in range(NCHUNK)]

            def local_layer(j):
                # entirely on the vector engine (+ sync ring for x) so the
                # scalar ring stays free for PE-paced evictions.
                for gt in range(GT):
                    xt = []
                    for l in range(L):
                        t = xpool.tile([128, NB], bf16, name=f"x{j}_{gt}_{l}", tag="x")
                        nc.sync.dma_start(
                            t[:],
                            x_p[l, gt * 128 : (gt + 1) * 128, j * NB : (j + 1) * NB],
                        )
                        xt.append(t)
                    acc = lpool.tile([128, NB], bf16, name=f"a{j}_{gt}_0", tag="acc")
                    nc.vector.tensor_scalar(
                        acc[:], xt[0][:], sc[gt][:, 0:1], None, mult
                    )
                    for l in range(1, L):
                        acc2 = lpool.tile(
                            [128, NB], bf16, name=f"a{j}_{gt}_{l}", tag="acc"
                        )
                        nc.vector.scalar_tensor_tensor(
                            acc2[:], xt[l][:], sc[gt][:, l : l + 1], acc[:], mult, add
                        )
                        acc = acc2
                    t = h0pool.tile([128, NB], bf16, name=f"h0_{j}_{gt}", tag="h0")
                    # relu(acc + b_local) on DVE
                    nc.vector.tensor_scalar(
                        t[:], acc[:], sc[gt][:, 4:5], 0.0, add, mybir.AluOpType.max
                    )
                    h0[j][gt] = t

            def load_w1_og(j, og):
                wb = []
                for k in range(GT):
                    w = wpool.tile([128, 1024], bf16, name=f"w1_{j}_{og}_{k}",
                                   tag="wblk")
                    nc.sync.dma_start(
                        w[:],
                        w1rp_p[k * 128 : (k + 1) * 128,
                               og * 1024 : (og + 1) * 1024],
                    )
                    wb.append(w)
                return wb

            def l1rp(j, preloaded=None):
                # row-parallel L1: partial[o, b] over own 1024 input feats.
                # og = 1024-wide output group; two half-og PSUM groups of 4
                # banks each so evictions overlap the next group's matmuls.
                for og in range(GT):
                    if preloaded is not None and og in preloaded:
                        wb = preloaded[og]
                    else:
                        wb = load_w1_og(j, og)
                    for half in range(2):
                        ps = [
                            ppool.tile([128, NB], fp32,
                                       name=f"ps1_{j}_{og}_{half}_{oo}", tag="ps")
                            for oo in range(4)
                        ]
                        for k in range(GT):
                            for oo in range(4):
                                ocol = half * 4 + oo
                                nc.tensor.matmul(
                                    ps[oo][:],
                                    wb[k][:, ocol * 128 : (ocol + 1) * 128],
                                    h0[j][k][:],
                                    start=(k == 0),
                                    stop=(k == GT - 1),
                                )
                        for oo in range(4):
                            t = epool.tile([128, NB], bf16,
                                           name=f"ev_{j}_{og}_{half}_{oo}", tag="ev")
                            nc.scalar.activation(t[:], ps[oo][:], Copy)
                            row = (og % 4) * 1024 + half * 512 + oo * 128
                            # part writes ride the scalar ring: they pace with
                            # the evict copies and never block weight prefetch
                            nc.scalar.dma_start(
                                part[j][og // 4][row : row + 128, :], t[:]
                            )

            def rs_half(j, a):
                nc.gpsimd.collective_compute(
                    "ReduceScatter", add, replica_groups=rg,
                    ins=[part[j][a][:].opt()],
                    outs=[rsout[j][a][:].opt()],
                )

            def relu_ag_half(j, a):
                # rsout[j][a] rows s -> own true feature a*512 + s.
                # Entirely on the vector engine + ring: these are the only ops
                # gated on collectives, so they must not sit on rings that
                # carry PE-feeding traffic.
                for t in range(4):
                    gt = a * 4 + t
                    tin = hpool.tile([128, NB], bf16, name=f"ri_{j}_{a}_{t}",
                                     tag="hin")
                    nc.gpsimd.dma_start(
                        tin[:], rsout[j][a][t * 128 : (t + 1) * 128, :]
                    )
                    tout = opool.tile([128, NB], bf16, name=f"ro_{j}_{a}_{t}",
                                      tag="hout")
                    nc.vector.tensor_scalar(
                        tout[:], tin[:], sc[gt][:, 5:6], 0.0, add,
                        mybir.AluOpType.max,
                    )
                    nc.gpsimd.dma_start(
                        slc1h[j][a][t * 128 : (t + 1) * 128, :], tout[:]
                    )
                nc.gpsimd.collective_compute(
                    "AllGather", bypass, replica_groups=rg,
                    ins=[slc1h[j][a][:].opt()],
                    outs=[gath1h[j][a][:].opt()],
                )

            def dense_layer(k, j):
                # k in {2,3}; k==2 input from gath1h halves, output slc2 -> AG;
                # k==3 input from gath2, output to out_p
                wt = w2t_p if k == 2 else w3t_p
                ps = [
                    ppool.tile([128, NB], fp32, name=f"ps{k}_{j}_{o}", tag="ps")
                    for o in range(GT)
                ]
                for g in range(KT):
                    ht = hpool.tile([128, NB], bf16, name=f"h{k}_{j}_{g}", tag="hin")
                    if k == 2:
                        hsrc = gath1h[j][g // (KT // 2)]
                        row = (g % (KT // 2)) * 128
                        nc.sync.dma_start(ht[:], hsrc[row : row + 128, :])
                    else:
                        nc.sync.dma_start(
                            ht[:], gath2[j][g * 128 : (g + 1) * 128, :]
                        )
                    wb = wpool.tile([128, GS], bf16, name=f"w{k}_{j}_{g}", tag="wblk")
                    nc.sync.dma_start(wb[:], wt[g * 128 : (g + 1) * 128, :])
                    for o in range(GT):
                        nc.tensor.matmul(
                            ps[o][:],
                            wb[:, o * 128 : (o + 1) * 128],
                            ht[:],
                            start=(g == 0),
                            stop=(g == KT - 1),
                        )
                for o in range(GT):
                    if k == 2:
                        ot = opool.tile(
                            [128, NB], bf16, name=f"o{k}_{j}_{o}", tag="hout"
                        )
                        nc.scalar.activation(
                            ot[:], ps[o][:], Relu, bias=sc[o][:, 6:7]
                        )
                        nc.scalar.dma_start(
                            slc2[j][o * 128 : (o + 1) * 128, :], ot[:]
                        )
                    else:
                        ot = opool.tile(
                            [128, NB], fp32, name=f"o{k}_{j}_{o}", tag="outp"
                        )
                        nc.scalar.activation(
                            ot[:], ps[o][:], Relu, bias=sc[o][:, 7:8]
                        )
                        nc.scalar.dma_start(
                            out_p[o * 128 : (o + 1) * 128, j * NB : (j + 1) * NB],
                            ot[:],
                        )

            # emission order = desired overlap order.  Preload L1c0's first
            # weight group ahead of the x stream so the PE starts ~12us in.
            pre = {0: load_w1_og(0, 0)}
            local_layer(0)
            l1rp(0, preloaded=pre)
            local_layer(1)
            rs_half(0, 0)
            relu_ag_half(0, 0)
            rs_half(0, 1)
            relu_ag_half(0, 1)
            l1rp(1)
            rs_half(1, 0)
            relu_ag_half(1, 0)
            rs_half(1, 1)
            relu_ag_half(1, 1)
            for j in range(NCHUNK):
                dense_layer(2, j)
                nc.gpsimd.collective_compute(
                    "AllGather", bypass, replica_groups=rg,
                    ins=[slc2[j][:].opt()],
                    outs=[gath2[j][:].opt()],
                )
            for j in range(NCHUNK):
                dense_layer(3, j)

    nc.compile()
    return nc


def _get_nc():
    if "nc" not in _compiled:
        _compiled["nc"] = _build_graph()
    return _compiled["nc"]


def kernel(x, W_local, b_local, W1, b1, W2, b2, W3, b3):
    from concourse.bass_utils import run_bass_kernel_spmd

    nc = _get_nc()

    x = np.asarray(x)
    W1p = np.asarray(W1)[_PERM_HALVES, :]      # rows = half-major outputs
    in_maps = []
    for r in range(N_CORES):
        sl = slice(r * GS, (r + 1) * GS)
        x_r = x[:, :, sl].transpose(0, 2, 1).astype(BF16)
        scal_r = np.concatenate(
            [
                np.asarray(W_local)[sl, :],
                np.asarray(b_local)[sl, None],
                np.asarray(b1)[sl, None],
                np.asarray(b2)[sl, None],
                np.asarray(b3)[sl, None],
            ],
            axis=1,
        ).astype(np.float32)
        in_maps.append(
            {
                "x": x_r,
                "scal": np.ascontiguousarray(scal_r),
                # [own 1024 in-feats, 8192 half-major out-feats]
                "w1rp": np.ascontiguousarray(W1p[:, sl].T).astype(BF16),
                # [8192 half-major in-feats, own 1024 out-feats]
                "w2t": np.asarray(W2)[sl, :].T.astype(BF16)[_PERM_HALVES, :],
                "w3t": np.asarray(W3)[sl, :].T.astype(BF16),
            }
        )

    res = run_bass_kernel_spmd(nc, in_maps, core_ids=list(range(N_CORES)))

    out = np.empty((B, G), np.float32)
    for r in range(N_CORES):
        out[:, r * GS : (r + 1) * GS] = res.results[r]["out"].T
    return out


# revision 19
# speedup vs baseline: 1.0646x; 1.0646x over previous
"""Trainium2 Bass kernel for nn_AMLNeuralNetwork3D (dense_mlp).

Strategy v4: layer 1 is ROW-parallel (each core contracts over its own
1024 input features, producing a [8192, batch] partial), so the PE can
start real matmuls ~12us into the kernel with ZERO preceding
communication -- the launch barrier and the first gather no longer gate
compute.  Partials are summed with bf16 AllReduce ops (feature halves);
every core then has the full pre-relu y1, and relu+b1 is applied
per-tile on the otherwise-idle vector engine as L2 streams its input.
Layers 2 and 3 stay column-parallel: L2 consumes the AllReduce halves
(W2 rows permuted into half-major order), AllGather, L3 consumes the
full gather and writes the per-core output slice.  Engine/ring
assignment is by dependency class so collective-gated ops never
head-of-line block PE-feeding traffic: sync ring carries only
monotonically-ready DMAs, the scalar ring carries PE-paced evictions
and output writes, the vector engine carries everything gated on
AllReduce results.

Activations stay feature-major [features, batch] on chip; weights are
pre-arranged on host so every DMA is wide/contiguous.  Compute in bf16
(fp32 PSUM accumulation).
"""

import sys

if "/opt/trn_rl_repo" not in sys.path:
    sys.path.insert(0, "/opt/trn_rl_repo")

import numpy as np
import ml_dtypes

N_CORES = 8
G = 8192          # genes / features
B = 1024          # batch
L = 4             # levels
GS = G // N_CORES # per-core feature slice (1024)
NB = 512          # batch chunk (one PSUM bank at fp32)
NCHUNK = B // NB  # 2
GT = GS // 128    # gene tiles per core slice (8)
KT = G // 128     # contraction tiles (64)

BF16 = ml_dtypes.bfloat16

_compiled = {}

# half-major feature order: [core0 f0:512, core1 f1024:1536, ...] then the
# second halves.  This is both the ReduceScatter row order of the L1
# partials and the AllGather'd h1 row order consumed by L2.
_PERM_HALVES = np.concatenate(
    [np.arange(r * 1024, r * 1024 + 512) for r in range(8)]
    + [np.arange(r * 1024 + 512, (r + 1) * 1024) for r in range(8)]
)


def _build_graph():
    from concourse import bacc, tile
    import concourse.mybir as mybir

    fp32 = mybir.dt.float32
    bf16 = mybir.dt.bfloat16
    Relu = mybir.ActivationFunctionType.Relu
    Copy = mybir.ActivationFunctionType.Copy
    mult = mybir.AluOpType.mult
    add = mybir.AluOpType.add
    bypass = mybir.AluOpType.bypass

    nc = bacc.Bacc(None, target_bir_lowering=False, num_devices=N_CORES)

    # ---- parameters (per-core shards; same graph on all cores) ----
    x_p = nc.declare_dram_parameter("x", [L, GS, B], bf16, isOutput=False)
    # per-feature scalars: cols 0..3 = W_local, 4 = b_local, 5..7 = b1..b3
    scal_p = nc.declare_dram_parameter("scal", [GS, 8], fp32, isOutput=False)
    # full b1 in half-major (_PERM_HALVES) order: [128 rows-in-ktile, 64 ktiles]
    b1p_p = nc.declare_dram_parameter("b1p", [128, KT], fp32, isOutput=False)
    # L1 row-parallel weights: [own 1024 in-feats, 8192 out-feats in
    # half-major (_PERM_HALVES) order]
    w1rp_p = nc.declare_dram_parameter("w1rp", [GS, G], bf16, isOutput=False)
    # L2: [8192 in-feats in half-major order, own 1024 out-feats]
    w2t_p = nc.declare_dram_parameter("w2t", [G, GS], bf16, isOutput=False)
    # L3: [8192 in-feats plain order, own 1024 out-feats]
    w3t_p = nc.declare_dram_parameter("w3t", [G, GS], bf16, isOutput=False)
    out_p = nc.declare_dram_parameter("out", [GS, B], fp32, isOutput=True)

    rg = [list(range(N_CORES))]

    with tile.TileContext(nc) as tc:
        with (
            tc.tile_pool(name="dram", bufs=1, space="DRAM") as dram,
            tc.tile_pool(name="scal", bufs=GT) as spool,
            tc.tile_pool(name="xin", bufs=16) as xpool,
            tc.tile_pool(name="loc", bufs=10) as lpool,
            tc.tile_pool(name="h0", bufs=16) as h0pool,
            tc.tile_pool(name="hin", bufs=40) as hpool,
            tc.tile_pool(name="wblk", bufs=24) as wpool,
            tc.tile_pool(name="evic", bufs=12) as epool,
            tc.tile_pool(name="hout", bufs=8) as opool,
            tc.tile_pool(name="psum", bufs=8, space="PSUM") as ppool,
        ):
            # L1 partial sums, per chunk, split in half-major halves
            part = [
                [
                    dram.tile([G // 2, NB], bf16, name=f"part_{j}_{a}",
                              tag=f"part_{j}_{a}")
                    for a in range(2)
                ]
                for j in range(NCHUNK)
            ]
            # AllReduce outputs: full summed pre-relu y1 halves on every core
            arout = [
                [
                    dram.tile([G // 2, NB], bf16, name=f"ar_{j}_{a}",
                              tag=f"ar_{j}_{a}", addr_space="Shared")
                    for a in range(2)
                ]
                for j in range(NCHUNK)
            ]
            # L2 -> L3 transition
            slc2 = [
                dram.tile([GS, NB], bf16, name=f"slc2_{j}", tag=f"slc2_{j}")
                for j in range(NCHUNK)
            ]
            gath2 = [
                dram.tile([G, NB], bf16, name=f"g2_{j}", tag=f"g2_{j}",
                          addr_space="Shared")
                for j in range(NCHUNK)
            ]

            # per-feature scalar tiles, persistent
            sc = []
            for gt in range(GT):
                s = spool.tile([128, 8], fp32, name=f"sc{gt}", tag="sc")
                nc.sync.dma_start(s[:], scal_p[gt * 128 : (gt + 1) * 128, :])
                sc.append(s)
            b1p = spool.tile([128, KT], fp32, name="b1p", tag="b1p")
            nc.sync.dma_start(b1p[:], b1p_p[:, :])

            h0 = [[None] * GT for _ in range(NCHUNK)]

            def local_layer(j):
                # entirely on the vector engine (+ sync ring for x) so the
                # scalar ring stays free for PE-paced evictions.
                for gt in range(GT):
                    xt = []
                    for l in range(L):
                        t = xpool.tile([128, NB], bf16, name=f"x{j}_{gt}_{l}", tag="x")
                        nc.sync.dma_start(
                            t[:],
                            x_p[l, gt * 128 : (gt + 1) * 128, j * NB : (j + 1) * NB],
                        )
                        xt.append(t)
                    acc = lpool.tile([128, NB], bf16, name=f"a{j}_{gt}_0", tag="acc")
                    nc.vector.tensor_scalar(
                        acc[:], xt[0][:], sc[gt][:, 0:1], None, mult
                    )
                    for l in range(1, L):
                        acc2 = lpool.tile(
                            [128, NB], bf16, name=f"a{j}_{gt}_{l}", tag="acc"
                        )
                        nc.vector.scalar_tensor_tensor(
                            acc2[:], xt[l][:], sc[gt][:, l : l + 1], acc[:], mult, add
                        )
                        acc = acc2
                    t = h0pool.tile([128, NB], bf16, name=f"h0_{j}_{gt}", tag="h0")
                    # relu(acc + b_local) on DVE
                    nc.vector.tensor_scalar(
                        t[:], acc[:], sc[gt][:, 4:5], 0.0, add, mybir.AluOpType.max
                    )
                    h0[j][gt] = t

            def load_w1_og(j, og):
                wb = []
                for k in range(GT):
                    w = wpool.tile([128, 1024], bf16, name=f"w1_{j}_{og}_{k}",
                                   tag="wblk")
                    nc.sync.dma_start(
                        w[:],
                        w1rp_p[k * 128 : (k + 1) * 128,
                               og * 1024 : (og + 1) * 1024],
                    )
                    wb.append(w)
                return wb

            def l1rp(j, preloaded=None):
                # row-parallel L1: partial[o, b] over own 1024 input feats.
                # og = 1024-wide output group; two half-og PSUM groups of 4
                # banks each so evictions overlap the next group's matmuls.
                for og in range(GT):
                    if preloaded is not None and og in preloaded:
                        wb = preloaded[og]
                    else:
                        wb = load_w1_og(j, og)
                    for half in range(2):
                        ps = [
                            ppool.tile([128, NB], fp32,
                                       name=f"ps1_{j}_{og}_{half}_{oo}", tag="ps")
                            for oo in range(4)
                        ]
                        for k in range(GT):
                            for oo in range(4):
                                ocol = half * 4 + oo
                                nc.tensor.matmul(
                                    ps[oo][:],
                                    wb[k][:, ocol * 128 : (ocol + 1) * 128],
                                    h0[j][k][:],
                                    start=(k == 0),
                                    stop=(k == GT - 1),
                                )
                        for oo in range(4):
                            t = epool.tile([128, NB], bf16,
                                           name=f"ev_{j}_{og}_{half}_{oo}", tag="ev")
                            nc.scalar.activation(t[:], ps[oo][:], Copy)
                            row = (og % 4) * 1024 + half * 512 + oo * 128
                            # part writes ride the scalar ring: they pace with
                            # the evict copies and never block weight prefetch
                            nc.scalar.dma_start(
                                part[j][og // 4][row : row + 128, :], t[:]
                            )

            def ar_half(j, a):
                nc.gpsimd.collective_compute(
                    "AllReduce", add, replica_groups=rg,
                    ins=[part[j][a][:].opt()],
                    outs=[arout[j][a][:].opt()],
                )

            def dense_layer(k, j):
                # k in {2,3}; k==2 input = relu(AllReduce'd y1) applied
                # per-tile on the vector engine; k==3 input from gath2,
                # output to out_p
                wt = w2t_p if k == 2 else w3t_p
                ps = [
                    ppool.tile([128, NB], fp32, name=f"ps{k}_{j}_{o}", tag="ps")
                    for o in range(GT)
                ]
                for g in range(KT):
                    if k == 2:
                        hsrc = arout[j][g // (KT // 2)]
                        row = (g % (KT // 2)) * 128
                        raw = hpool.tile([128, NB], bf16, name=f"r{k}_{j}_{g}",
                                         tag="hin")
                        nc.sync.dma_start(raw[:], hsrc[row : row + 128, :])
                        ht = hpool.tile([128, NB], bf16, name=f"h{k}_{j}_{g}",
                                        tag="hin")
                        nc.vector.tensor_scalar(
                            ht[:], raw[:], b1p[:, g : g + 1], 0.0, add,
                            mybir.AluOpType.max,
                        )
                    else:
                        ht = hpool.tile([128, NB], bf16, name=f"h{k}_{j}_{g}",
                                        tag="hin")
                        nc.sync.dma_start(
                            ht[:], gath2[j][g * 128 : (g + 1) * 128, :]
                        )
                    wb = wpool.tile([128, GS], bf16, name=f"w{k}_{j}_{g}", tag="wblk")
                    nc.sync.dma_start(wb[:], wt[g * 128 : (g + 1) * 128, :])
                    for o in range(GT):
                        nc.tensor.matmul(
                            ps[o][:],
                            wb[:, o * 128 : (o + 1) * 128],
                            ht[:],
                            start=(g == 0),
                            stop=(g == KT - 1),
                        )
                for o in range(GT):
                    if k == 2:
                        ot = opool.tile(
                            [128, NB], bf16, name=f"o{k}_{j}_{o}", tag="hout"
                        )
                        nc.scalar.activation(
                            ot[:], ps[o][:], Relu, bias=sc[o][:, 6:7]
                        )
                        nc.scalar.dma_start(
                            slc2[j][o * 128 : (o + 1) * 128, :], ot[:]
                        )
                    else:
                        ot = opool.tile(
                            [128, NB], fp32, name=f"o{k}_{j}_{o}", tag="outp"
                        )
                        nc.scalar.activation(
                            ot[:], ps[o][:], Relu, bias=sc[o][:, 7:8]
                        )
                        nc.scalar.dma_start(
                            out_p[o * 128 : (o + 1) * 128, j * NB : (j + 1) * NB],
                            ot[:],
                        )

            # emission order = desired overlap order.  Preload L1c0's first
            # weight group ahead of the x stream so the PE starts ~12us in.
            pre = {0: load_w1_og(0, 0)}
            local_layer(0)
            l1rp(0, preloaded=pre)
            local_layer(1)
            ar_half(0, 0)
            ar_half(0, 1)
            l1rp(1)
            ar_half(1, 0)
            ar_half(1, 1)
            for j in range(NCHUNK):
                dense_layer(2, j)
                nc.gpsimd.collective_compute(
                    "AllGather", bypass, replica_groups=rg,
                    ins=[slc2[j][:].opt()],
                    outs=[gath2[j][:].opt()],
                )
            for j in range(NCHUNK):
                dense_layer(3, j)

    nc.compile()
    return nc


def _get_nc():
    if "nc" not in _compiled:
        _compiled["nc"] = _build_graph()
    return _compiled["nc"]


def kernel(x, W_local, b_local, W1, b1, W2, b2, W3, b3):
    from concourse.bass_utils import run_bass_kernel_spmd

    nc = _get_nc()

    x = np.asarray(x)
    W1p = np.asarray(W1)[_PERM_HALVES, :]      # rows = half-major outputs
    # b1 in half-major order as [128 rows-in-ktile, 64 ktiles]
    b1p = np.ascontiguousarray(
        np.asarray(b1)[_PERM_HALVES].reshape(KT, 128).T
    ).astype(np.float32)
    in_maps = []
    for r in range(N_CORES):
        sl = slice(r * GS, (r + 1) * GS)
        x_r = x[:, :, sl].transpose(0, 2, 1).astype(BF16)
        scal_r = np.concatenate(
            [
                np.asarray(W_local)[sl, :],
                np.asarray(b_local)[sl, None],
                np.asarray(b1)[sl, None],
                np.asarray(b2)[sl, None],
                np.asarray(b3)[sl, None],
            ],
            axis=1,
        ).astype(np.float32)
        in_maps.append(
            {
                "x": x_r,
                "scal": np.ascontiguousarray(scal_r),
                "b1p": b1p,
                # [own 1024 in-feats, 8192 half-major out-feats]
                "w1rp": np.ascontiguousarray(W1p[:, sl].T).astype(BF16),
                # [8192 half-major in-feats, own 1024 out-feats]
                "w2t": np.asarray(W2)[sl, :].T.astype(BF16)[_PERM_HALVES, :],
                "w3t": np.asarray(W3)[sl, :].T.astype(BF16),
            }
        )

    res = run_bass_kernel_spmd(nc, in_maps, core_ids=list(range(N_CORES)))

    out = np.empty((B, G), np.float32)
    for r in range(N_CORES):
        out[:, r * GS : (r + 1) * GS] = res.results[r]["out"].T
    return out


# revision 25
# speedup vs baseline: 1.0814x; 1.0158x over previous
"""Trainium2 Bass kernel for nn_AMLNeuralNetwork3D (dense_mlp).

Strategy v4: layer 1 is ROW-parallel (each core contracts over its own
1024 input features, producing a [8192, batch] partial), so the PE can
start real matmuls ~12us into the kernel with ZERO preceding
communication -- the launch barrier and the first gather no longer gate
compute.  Partials are summed with bf16 AllReduce ops (feature halves);
every core then has the full pre-relu y1, and relu+b1 is applied
per-tile on the otherwise-idle vector engine as L2 streams its input.
Layers 2 and 3 stay column-parallel: L2 consumes the AllReduce halves
(W2 rows permuted into half-major order), AllGather, L3 consumes the
full gather and writes the per-core output slice.  Engine/ring
assignment is by dependency class so collective-gated ops never
head-of-line block PE-feeding traffic: sync ring carries only
monotonically-ready DMAs, the scalar ring carries PE-paced evictions
and output writes, the vector engine carries everything gated on
AllReduce results.

Activations stay feature-major [features, batch] on chip; weights are
pre-arranged on host so every DMA is wide/contiguous.  Compute in bf16
(fp32 PSUM accumulation).
"""

import sys

if "/opt/trn_rl_repo" not in sys.path:
    sys.path.insert(0, "/opt/trn_rl_repo")

import numpy as np
import ml_dtypes

N_CORES = 8
G = 8192          # genes / features
B = 1024          # batch
L = 4             # levels
GS = G // N_CORES # per-core feature slice (1024)
NB = 512          # batch chunk (one PSUM bank at fp32)
NCHUNK = B // NB  # 2
GT = GS // 128    # gene tiles per core slice (8)
KT = G // 128     # contraction tiles (64)

BF16 = ml_dtypes.bfloat16

_compiled = {}

# half-major feature order: [core0 f0:512, core1 f1024:1536, ...] then the
# second halves.  This is both the ReduceScatter row order of the L1
# partials and the AllGather'd h1 row order consumed by L2.
_PERM_HALVES = np.concatenate(
    [np.arange(r * 1024, r * 1024 + 512) for r in range(8)]
    + [np.arange(r * 1024 + 512, (r + 1) * 1024) for r in range(8)]
)


def _build_graph():
    from concourse import bacc, tile
    import concourse.mybir as mybir

    fp32 = mybir.dt.float32
    bf16 = mybir.dt.bfloat16
    Relu = mybir.ActivationFunctionType.Relu
    Copy = mybir.ActivationFunctionType.Copy
    mult = mybir.AluOpType.mult
    add = mybir.AluOpType.add
    bypass = mybir.AluOpType.bypass

    nc = bacc.Bacc(None, target_bir_lowering=False, num_devices=N_CORES)

    # ---- parameters (per-core shards; same graph on all cores) ----
    x_p = nc.declare_dram_parameter("x", [L, GS, B], bf16, isOutput=False)
    # per-feature scalars: cols 0..3 = W_local, 4 = b_local, 5..7 = b1..b3
    scal_p = nc.declare_dram_parameter("scal", [GS, 8], fp32, isOutput=False)
    # full b1 in half-major (_PERM_HALVES) order: [128 rows-in-ktile, 64 ktiles]
    b1p_p = nc.declare_dram_parameter("b1p", [128, KT], fp32, isOutput=False)
    # L1 row-parallel weights: [own 1024 in-feats, 8192 out-feats in
    # half-major (_PERM_HALVES) order]
    w1rp_p = nc.declare_dram_parameter("w1rp", [GS, G], bf16, isOutput=False)
    # L2: [8192 in-feats in half-major order, own 1024 out-feats]
    w2t_p = nc.declare_dram_parameter("w2t", [G, GS], bf16, isOutput=False)
    # L3: [8192 in-feats plain order, own 1024 out-feats]
    w3t_p = nc.declare_dram_parameter("w3t", [G, GS], bf16, isOutput=False)
    out_p = nc.declare_dram_parameter("out", [GS, B], fp32, isOutput=True)

    rg = [list(range(N_CORES))]

    with tile.TileContext(nc) as tc:
        with (
            tc.tile_pool(name="dram", bufs=1, space="DRAM") as dram,
            tc.tile_pool(name="scal", bufs=GT) as spool,
            tc.tile_pool(name="xin", bufs=16) as xpool,
            tc.tile_pool(name="loc", bufs=10) as lpool,
            tc.tile_pool(name="h0", bufs=16) as h0pool,
            tc.tile_pool(name="hin", bufs=40) as hpool,
            tc.tile_pool(name="wblk", bufs=24) as wpool,
            tc.tile_pool(name="evic", bufs=36) as epool,
            tc.tile_pool(name="hout", bufs=8) as opool,
            tc.tile_pool(name="psum", bufs=8, space="PSUM") as ppool,
        ):
            # L1 partial sums.  Chunk 0 is split in half-major halves (early
            # first delivery under launch skew); chunk 1 is one buffer (its
            # AllReduce runs as a single op -- shorter serial CC chain).
            part0 = [
                dram.tile([G // 2, NB], bf16, name=f"part_0_{a}",
                          tag=f"part_0_{a}")
                for a in range(2)
            ]
            part1 = dram.tile([G, NB], bf16, name="part_1", tag="part_1")
            arout0 = [
                dram.tile([G // 2, NB], bf16, name=f"ar_0_{a}",
                          tag=f"ar_0_{a}", addr_space="Shared")
                for a in range(2)
            ]
            arout1 = dram.tile([G, NB], bf16, name="ar_1", tag="ar_1",
                               addr_space="Shared")
            # L2 -> L3 transition
            slc2 = [
                dram.tile([GS, NB], bf16, name=f"slc2_{j}", tag=f"slc2_{j}")
                for j in range(NCHUNK)
            ]
            gath2 = [
                dram.tile([G, NB], bf16, name=f"g2_{j}", tag=f"g2_{j}",
                          addr_space="Shared")
                for j in range(NCHUNK)
            ]

            # per-feature scalar tiles, persistent
            sc = []
            for gt in range(GT):
                s = spool.tile([128, 8], fp32, name=f"sc{gt}", tag="sc")
                nc.sync.dma_start(s[:], scal_p[gt * 128 : (gt + 1) * 128, :])
                sc.append(s)
            b1p = spool.tile([128, KT], fp32, name="b1p", tag="b1p")
            nc.sync.dma_start(b1p[:], b1p_p[:, :])

            h0 = [[None] * GT for _ in range(NCHUNK)]

            def local_layer(j):
                # entirely on the vector engine (+ sync ring for x) so the
                # scalar ring stays free for PE-paced evictions.
                for gt in range(GT):
                    xt = []
                    for l in range(L):
                        t = xpool.tile([128, NB], bf16, name=f"x{j}_{gt}_{l}", tag="x")
                        nc.sync.dma_start(
                            t[:],
                            x_p[l, gt * 128 : (gt + 1) * 128, j * NB : (j + 1) * NB],
                        )
                        xt.append(t)
                    acc = lpool.tile([128, NB], bf16, name=f"a{j}_{gt}_0", tag="acc")
                    nc.vector.tensor_scalar(
                        acc[:], xt[0][:], sc[gt][:, 0:1], None, mult
                    )
                    for l in range(1, L):
                        acc2 = lpool.tile(
                            [128, NB], bf16, name=f"a{j}_{gt}_{l}", tag="acc"
                        )
                        nc.vector.scalar_tensor_tensor(
                            acc2[:], xt[l][:], sc[gt][:, l : l + 1], acc[:], mult, add
                        )
                        acc = acc2
                    t = h0pool.tile([128, NB], bf16, name=f"h0_{j}_{gt}", tag="h0")
                    # relu(acc + b_local) on DVE
                    nc.vector.tensor_scalar(
                        t[:], acc[:], sc[gt][:, 4:5], 0.0, add, mybir.AluOpType.max
                    )
                    h0[j][gt] = t

            def load_w1_og(j, og):
                wb = []
                for k in range(GT):
                    w = wpool.tile([128, 1024], bf16, name=f"w1_{j}_{og}_{k}",
                                   tag="wblk")
                    nc.sync.dma_start(
                        w[:],
                        w1rp_p[k * 128 : (k + 1) * 128,
                               og * 1024 : (og + 1) * 1024],
                    )
                    wb.append(w)
                return wb

            def l1rp(j, preloaded=None):
                # row-parallel L1: partial[o, b] over own 1024 input feats.
                # og = 1024-wide output group; two half-og PSUM groups of 4
                # banks each so evictions overlap the next group's matmuls.
                for og in range(GT):
                    if preloaded is not None and og in preloaded:
                        wb = preloaded[og]
                    else:
                        wb = load_w1_og(j, og)
                    for half in range(2):
                        ps = [
                            ppool.tile([128, NB], fp32,
                                       name=f"ps1_{j}_{og}_{half}_{oo}", tag="ps")
                            for oo in range(4)
                        ]
                        for k in range(GT):
                            for oo in range(4):
                                ocol = half * 4 + oo
                                nc.tensor.matmul(
                                    ps[oo][:],
                                    wb[k][:, ocol * 128 : (ocol + 1) * 128],
                                    h0[j][k][:],
                                    start=(k == 0),
                                    stop=(k == GT - 1),
                                )
                        for oo in range(4):
                            t = epool.tile([128, NB], bf16,
                                           name=f"ev_{j}_{og}_{half}_{oo}", tag="ev")
                            nc.scalar.activation(t[:], ps[oo][:], Copy)
                            # part writes ride the scalar ring: they pace with
                            # the evict copies and never block weight prefetch
                            if j == 0:
                                row = (og % 4) * 1024 + half * 512 + oo * 128
                                nc.scalar.dma_start(
                                    part0[og // 4][row : row + 128, :], t[:]
                                )
                            else:
                                row = og * 1024 + half * 512 + oo * 128
                                nc.scalar.dma_start(
                                    part1[row : row + 128, :], t[:]
                                )

            def ar0_half(a):
                nc.gpsimd.collective_compute(
                    "AllReduce", add, replica_groups=rg,
                    ins=[part0[a][:].opt()],
                    outs=[arout0[a][:].opt()],
                )

            def ar1_full():
                nc.gpsimd.collective_compute(
                    "AllReduce", add, replica_groups=rg,
                    ins=[part1[:].opt()],
                    outs=[arout1[:].opt()],
                )

            def dense_layer(k, j):
                # k in {2,3}; k==2 input = relu(AllReduce'd y1) applied
                # per-tile on the vector engine; k==3 input from gath2,
                # output to out_p
                wt = w2t_p if k == 2 else w3t_p
                ps = [
                    ppool.tile([128, NB], fp32, name=f"ps{k}_{j}_{o}", tag="ps")
                    for o in range(GT)
                ]
                for g in range(KT):
                    if k == 2:
                        raw = hpool.tile([128, NB], bf16, name=f"r{k}_{j}_{g}",
                                         tag="hin")
                        if j == 0:
                            hsrc = arout0[g // (KT // 2)]
                            row = (g % (KT // 2)) * 128
                        else:
                            hsrc = arout1
                            row = g * 128
                        nc.sync.dma_start(raw[:], hsrc[row : row + 128, :])
                        ht = hpool.tile([128, NB], bf16, name=f"h{k}_{j}_{g}",
                                        tag="hin")
                        nc.vector.tensor_scalar(
                            ht[:], raw[:], b1p[:, g : g + 1], 0.0, add,
                            mybir.AluOpType.max,
                        )
                    else:
                        ht = hpool.tile([128, NB], bf16, name=f"h{k}_{j}_{g}",
                                        tag="hin")
                        nc.sync.dma_start(
                            ht[:], gath2[j][g * 128 : (g + 1) * 128, :]
                        )
                    wb = wpool.tile([128, GS], bf16, name=f"w{k}_{j}_{g}", tag="wblk")
                    nc.sync.dma_start(wb[:], wt[g * 128 : (g + 1) * 128, :])
                    for o in range(GT):
                        nc.tensor.matmul(
                            ps[o][:],
                            wb[:, o * 128 : (o + 1) * 128],
                            ht[:],
                            start=(g == 0),
                            stop=(g == KT - 1),
                        )
                for o in range(GT):
                    if k == 2:
                        ot = opool.tile(
                            [128, NB], bf16, name=f"o{k}_{j}_{o}", tag="hout"
                        )
                        nc.scalar.activation(
                            ot[:], ps[o][:], Relu, bias=sc[o][:, 6:7]
                        )
                        nc.scalar.dma_start(
                            slc2[j][o * 128 : (o + 1) * 128, :], ot[:]
                        )
                    else:
                        ot = opool.tile(
                            [128, NB], fp32, name=f"o{k}_{j}_{o}", tag="outp"
                        )
                        nc.scalar.activation(
                            ot[:], ps[o][:], Relu, bias=sc[o][:, 7:8]
                        )
                        nc.scalar.dma_start(
                            out_p[o * 128 : (o + 1) * 128, j * NB : (j + 1) * NB],
                            ot[:],
                        )

            # emission order = desired overlap order.  Preload L1c0's first
            # weight group ahead of the x stream so the PE starts ~12us in.
            pre = {0: load_w1_og(0, 0)}
            local_layer(0)
            l1rp(0, preloaded=pre)
            local_layer(1)
            ar0_half(0)
            ar0_half(1)
            l1rp(1)
            ar1_full()
            for j in range(NCHUNK):
                dense_layer(2, j)
                nc.gpsimd.collective_compute(
                    "AllGather", bypass, replica_groups=rg,
                    ins=[slc2[j][:].opt()],
                    outs=[gath2[j][:].opt()],
                )
            for j in range(NCHUNK):
                dense_layer(3, j)

    nc.compile()
    return nc


def _get_nc():
    if "nc" not in _compiled:
        _compiled["nc"] = _build_graph()
    return _compiled["nc"]


def kernel(x, W_local, b_local, W1, b1, W2, b2, W3, b3):
    from concourse.bass_utils import run_bass_kernel_spmd

    nc = _get_nc()

    x = np.asarray(x)
    W1p = np.asarray(W1)[_PERM_HALVES, :]      # rows = half-major outputs
    # b1 in half-major order as [128 rows-in-ktile, 64 ktiles]
    b1p = np.ascontiguousarray(
        np.asarray(b1)[_PERM_HALVES].reshape(KT, 128).T
    ).astype(np.float32)
    in_maps = []
    for r in range(N_CORES):
        sl = slice(r * GS, (r + 1) * GS)
        x_r = x[:, :, sl].transpose(0, 2, 1).astype(BF16)
        scal_r = np.concatenate(
            [
                np.asarray(W_local)[sl, :],
                np.asarray(b_local)[sl, None],
                np.asarray(b1)[sl, None],
                np.asarray(b2)[sl, None],
                np.asarray(b3)[sl, None],
            ],
            axis=1,
        ).astype(np.float32)
        in_maps.append(
            {
                "x": x_r,
                "scal": np.ascontiguousarray(scal_r),
                "b1p": b1p,
                # [own 1024 in-feats, 8192 half-major out-feats]
                "w1rp": np.ascontiguousarray(W1p[:, sl].T).astype(BF16),
                # [8192 half-major in-feats, own 1024 out-feats]
                "w2t": np.asarray(W2)[sl, :].T.astype(BF16)[_PERM_HALVES, :],
                "w3t": np.asarray(W3)[sl, :].T.astype(BF16),
            }
        )

    res = run_bass_kernel_spmd(nc, in_maps, core_ids=list(range(N_CORES)))

    out = np.empty((B, G), np.float32)
    for r in range(N_CORES):
        out[:, r * GS : (r + 1) * GS] = res.results[r]["out"].T
    return out
